# revision 50
# baseline (speedup 1.0000x reference)
"""Multi-head causal attention (B=2, L=2048, E=1024, H=16, D=64) on 8 NeuronCores.

Sharding: data-parallel over batch x tensor-parallel over heads.
  core c: batch b = c // 4, head group hg = c % 4 -> heads [4*hg, 4*hg+4).
Each core computes QKV projection for its 4 heads, causal softmax attention,
and a *partial* output projection (its heads' slice of Wout). The host sums
the 4 partial outputs per batch and adds the bias.

Device design notes (v2):
  - Matmul operands are bf16 (fp32 PSUM accumulation); host pre-casts inputs.
  - Host pre-transposes everything so the device never transposes:
      xT   [E, L]   = x[b].T                      (bf16)
      waT  [E, 768] = Wa rows regrouped [q_h0..q_h3 | k_h0.. | v_h0..].T (bf16)
      woT  [256, E] = Wout_w columns for this core's heads, transposed   (bf16)
  - Attention runs in the S^T layout (scores[j, i]); softmax denominator Z
    comes from a ones-column appended to V (PSUM row 64; row 65 is pad).
  - No max-subtraction in softmax: scores are ~N(0, 0.41^2), exp can't overflow.
  - The attention middle is ScalarE(exp)-bound (~72us), so all other PE work
    (second-half QKV projection, output projection) is drip-fed into the
    attention stream via a cycle-budget greedy to keep the PE dense and the
    HAM clock-gate at 8/8. A warmup matmul burst during the input DMA trips
    the HAM to full clock before the QKV projection starts.
  - ScalarE runs ONLY exp (plus 4 early copies before the first exp); all
    other PSUM->SBUF moves go to VectorE, the causal-mask multiplies go to
    GpSimd, 1/Z is a direct-from-PSUM fast reciprocal, and the per-column
    1/Z broadcast is an SBUF->SBUF partition-broadcast DMA (no PE/DVE cost).
  - Output projection accumulates both head-pairs in one PSUM tile (no
    HBM read-modify-write accumulation, half the output DMA).
"""

import ml_dtypes
import numpy as np

import concourse.bass as bass
import concourse.mybir as mybir
import concourse.tile as tile
from concourse import bacc
from concourse.bass_utils import run_bass_kernel_spmd
from concourse.masks import make_upper_triangular


P = 128
B = 2
L = 2048
E = 1024
H = 16
D = 64
HC = 4            # heads per core
F = HC * D        # 256: this core's slice of the head dim
EC = E // P       # 8 chunks of the embed dim
NLC = L // P      # 16 l-chunks
VST = NLC * 66    # v stride per head: 16 chunks of [64 v | 1 ones | 1 pad]

f32 = mybir.dt.float32
f32r = mybir.dt.float32r
bf16 = mybir.dt.bfloat16
AF = mybir.ActivationFunctionType
N_CORES = 8


DEBUG_TAPS = False


def build_nc():
    nc = bacc.Bacc(None, target_bir_lowering=False, debug=False)

    xT = nc.dram_tensor("xT", [E, L], bf16, kind="ExternalInput")
    waT = nc.dram_tensor("waT", [E, 3 * F], bf16, kind="ExternalInput")
    woT = nc.dram_tensor("woT", [F, E], bf16, kind="ExternalInput")
    outT = nc.dram_tensor("outT", [E, L], bf16, kind="ExternalOutput")
    if DEBUG_TAPS:
        dbg = {
            "qTo": nc.dram_tensor("qTo", [2 * P, L], bf16, kind="ExternalOutput"),
            "kTo": nc.dram_tensor("kTo", [2 * P, L], bf16, kind="ExternalOutput"),
            "vono": nc.dram_tensor(
                "vono", [P, HC * VST], bf16, kind="ExternalOutput"
            ),
            "oTo": nc.dram_tensor("oTo", [2 * P, L], bf16, kind="ExternalOutput"),
        }

    with tile.TileContext(nc) as tc:
        with (
            tc.tile_pool(name="persist", bufs=1) as pp,
            tc.tile_pool(name="qkv", bufs=1) as qp,
            tc.tile_pool(name="xw", bufs=1) as xp,
            tc.tile_pool(name="sps", bufs=4, space="PSUM") as sp,
            tc.tile_pool(name="ops", bufs=4, space="PSUM") as op_,
            tc.tile_pool(name="epool", bufs=8) as ep,
            tc.tile_pool(name="npool", bufs=2) as npl,
            tc.tile_pool(name="ob", bufs=3) as ob,
        ):
            # Persistent SBUF tensors.
            qT = [qp.tile([P, L], bf16, tag=f"q{p}", name=f"qT{p}") for p in range(2)]
            kT = [qp.tile([P, L], bf16, tag=f"k{p}", name=f"kT{p}") for p in range(2)]
            von = qp.tile([P, HC * VST], bf16, tag="von", name="von")
            oT = [qp.tile([P, L], bf16, tag=f"o{p}", name=f"oT{p}") for p in range(2)]
            wo_sb = [
                pp.tile([P, E], bf16, tag=f"wo{fc}", name=f"wo{fc}") for fc in range(2)
            ]
            onesf = pp.tile([P, 64], f32, tag="onesf")
            ones1 = pp.tile([1, 64], f32r, tag="ones1")
            warm_w = pp.tile([P, 64], bf16, tag="warm")
            trimask = pp.tile([P, P], bf16, tag="trimask")
            trimaskf = pp.tile([P, P], f32, tag="trimaskf")

            x_sb = [
                xp.tile([P, L], bf16, tag=f"x{ec}", name=f"x{ec}") for ec in range(EC)
            ]
            wa_sb = [
                xp.tile([P, 3 * F], bf16, tag=f"wa{ec}", name=f"wa{ec}")
                for ec in range(EC)
            ]

            # Input DMAs in priority order: the l-half-0 x plus the q/k
            # sections of Wa unblock the first QKV tiles ~10us in; v/x-lb1/wo
            # stream behind.
            for ec in range(EC):
                nc.sync.dma_start(x_sb[ec][:, 0:1024], xT[ec * P : (ec + 1) * P, 0:1024])
                nc.sync.dma_start(
                    wa_sb[ec][:, 0 : 2 * F], waT[ec * P : (ec + 1) * P, 0 : 2 * F]
                )
            for ec in range(EC):
                nc.sync.dma_start(
                    wa_sb[ec][:, 2 * F : 3 * F],
                    waT[ec * P : (ec + 1) * P, 2 * F : 3 * F],
                )
            for ec in range(EC):
                nc.sync.dma_start(
                    x_sb[ec][:, 1024:2048], xT[ec * P : (ec + 1) * P, 1024:2048]
                )
            for fc in range(2):
                nc.sync.dma_start(wo_sb[fc][:], woT[fc * P : (fc + 1) * P, :])

            # Constants (memset/affine_select can't encode bf16: build f32, cast)
            nc.gpsimd.memset(onesf[:], 1.0)
            nc.vector.tensor_copy(warm_w[:], onesf[:])
            nc.vector.tensor_copy(ones1[:], onesf[0:1, :])
            make_upper_triangular(nc, trimaskf[:], val=1.0, diag=True)
            nc.vector.tensor_copy(trimask[:], trimaskf[:])
            # ones/pad columns of von (Z rows): cols [64:66] of each 66-chunk
            for h in range(HC):
                dst = von[:].rearrange("p (g n t) -> p g n t", g=HC, t=66)[
                    :, h, :, 64:66
                ]
                nc.vector.tensor_copy(
                    dst, onesf[:, 0:32].rearrange("p (n t) -> p n t", t=2)
                )

            # HAM warmup: ~7us of junk matmuls spanning the input-DMA wait, so
            # the PE clock-gate is at 8/8 when the real work starts and never
            # sees a >3.4us idle window at the kernel head.
            wps = sp.tile([P, 512], f32, tag="ps")
            for i in range(128):
                nc.tensor.matmul(
                    wps[0:64, 0:64], warm_w[:], warm_w[:], start=True, stop=True
                )

            # ---------------- emitters ----------------
            est = {"pe": 0.0, "act": 0.0}

            def emit_qk_half(p, which, lb, s, eng):
                off = 0 if which == "q" else F
                dst = (qT if which == "q" else kT)[p]
                cp = nc.scalar.copy if eng == "s" else nc.vector.tensor_copy
                ps = sp.tile([P, 512], f32, tag="ps")
                for ec in range(EC):
                    nc.tensor.matmul(
                        ps[:],
                        wa_sb[ec][:, off + p * P : off + (p + 1) * P],
                        x_sb[ec][:, lb * 1024 + s * 512 : lb * 1024 + (s + 1) * 512],
                        start=(ec == 0),
                        stop=(ec == EC - 1),
                    )
                cp(dst[:, lb * 1024 + s * 512 : lb * 1024 + (s + 1) * 512], ps[:])
                est["pe"] += 1707

            def emit_qk(p, which, lb, eng):
                for s in range(2):
                    emit_qk_half(p, which, lb, s, eng)

            def emit_v(lc, eng):
                # v natural [l, d] for all 4 heads at once (free dim 256)
                ps = sp.tile([P, 512], f32, tag="ps")
                for ec in range(EC):
                    nc.tensor.matmul(
                        ps[:, 0:F],
                        x_sb[ec][:, lc * P : (lc + 1) * P],
                        wa_sb[ec][:, 2 * F : 3 * F],
                        start=(ec == 0),
                        stop=(ec == EC - 1),
                    )
                dst = von[:].rearrange("p (g c) -> p g c", g=HC)[
                    :, :, lc * 66 : lc * 66 + 64
                ]
                src = ps[:, 0:F].rearrange("p (g c) -> p g c", g=HC)
                cp = nc.scalar.copy if eng == "s" else nc.vector.tensor_copy
                cp(dst, src)
                est["pe"] += 853

            def emit_av(u):
                # One chunk = one 512-aligned block of i: a single matmul
                # into the [hl][half] po accumulator.
                h, po_half, et, jc, off, cw, qg = u
                nc.tensor.matmul(
                    po_half[:, off : off + cw],
                    von[:, h * VST + jc * 66 : h * VST + (jc + 1) * 66],
                    et[:, 0:cw],
                    start=(jc == 0),
                    stop=(jc == 4 * qg + 3),
                )

            def emit_norm1(po2, half):
                # phase 1 of normalization for one 512-wide i-half: 1/Z into
                # SBUF (VectorE). reciprocal_approx_fast reads garbage from
                # PSUM on HW: bounce the Z row through SBUF first.
                zfs = []
                for hl in range(2):
                    zc = npl.tile([1, 512], f32, tag="zc")
                    nc.vector.tensor_copy(zc[:], po2[hl][half][64:65, :])
                    zf = npl.tile([1, 512], f32, tag="zf")
                    nc.vector.reciprocal_approx_fast(zf[:], zc[:])
                    zfs.append(zf)
                return zfs

            def emit_norm2(p, sweep, half, po2, zfs):
                # phase 2: replicate 1/Z across 64 partitions with a K=1
                # ones-outer-product matmul (PE; partition_broadcast on
                # GpSimd forces a ~6us Q7 library swap around every call),
                # bounce to SBUF, and apply.
                win = slice(
                    sweep * 1024 + half * 512, sweep * 1024 + (half + 1) * 512
                )
                for hl in range(2):
                    zr = npl.tile([1, 512], f32r, tag="zr")
                    nc.vector.tensor_copy(zr[:], zfs[hl][:])
                    zp = sp.tile([P, 512], f32, tag="ps")
                    nc.tensor.matmul(
                        zp[0:64, 0:512], ones1[:], zr[:], start=True, stop=True
                    )
                    zs = npl.tile([64, 512], f32, tag="zs")
                    nc.vector.tensor_copy(zs[:], zp[0:64, 0:512])
                    nc.vector.tensor_mul(
                        oT[p][hl * 64 : (hl + 1) * 64, win],
                        po2[hl][half][0:64, :],
                        zs[:],
                    )

            def emit_oproj(oc, lb, eng):
                # output projection for out-chunk oc, l-half lb: both head
                # pairs accumulate in one PSUM tile (K=128 each).
                cp = nc.scalar.copy if eng == "s" else nc.vector.tensor_copy
                ot = ob.tile([P, 1024], bf16, tag="ot")
                for s in range(2):
                    ps = sp.tile([P, 512], f32, tag="ps")
                    for fc in range(2):
                        nc.tensor.matmul(
                            ps[:],
                            wo_sb[fc][:, oc * P : (oc + 1) * P],
                            oT[fc][:, lb * 1024 + s * 512 : lb * 1024 + (s + 1) * 512],
                            start=(fc == 0),
                            stop=(fc == 1),
                        )
                    cp(ot[:, s * 512 : (s + 1) * 512], ps[:])
                nc.sync.dma_start(
                    outT[oc * P : (oc + 1) * P, lb * 1024 : (lb + 1) * 1024], ot[:]
                )
                est["pe"] += 853

            # ---------------- schedule ----------------
            # Opening: only the first 512-wide q/k halves of pair 0 precede
            # the first attention chunk; the second halves are emitted inside
            # unit (0,0,0) between its two chunks (copies on the still-idle
            # ScalarE). Everything else is a filler.
            emit_qk_half(0, "q", 0, 0, "s")
            emit_qk_half(0, "k", 0, 0, "s")

            # Filler list (order matters: consumed FIFO; barriers below index
            # into it). All remaining QKV + (appended later) the first-half
            # output projection.
            fillers = []
            fillers += [lambda lc=lc: emit_v(lc, "v") for lc in range(0, 8)]  # 0..7
            fillers += [
                lambda: emit_qk(1, "q", 0, "v"),  # 8
                lambda: emit_qk(1, "k", 0, "v"),  # 9
                lambda: emit_qk(0, "q", 1, "v"),  # 10
                lambda: emit_qk(0, "k", 1, "v"),  # 11
            ]
            fillers += [lambda lc=lc: emit_v(lc, "v") for lc in range(8, 16)]  # 12..19
            fillers += [
                lambda: emit_qk(1, "q", 1, "v"),  # 20
                lambda: emit_qk(1, "k", 1, "v"),  # 21
            ]
            drained = {"n": 0}

            def drain_until(k):
                while drained["n"] < k:
                    fillers[drained["n"]]()
                    drained["n"] += 1

            reserve = [6]  # fillers held back for the final (filler-less) sweep

            def drain_budget():
                # Emit fillers while the PE would otherwise idle behind ACT,
                # but at most 2 per unit so the supply lasts to the final
                # (pure-attention) sweeps instead of front-loading.
                n = 0
                while (
                    drained["n"] < len(fillers) - reserve[0]
                    and est["act"] - est["pe"] > 400
                    and n < 2
                ):
                    fillers[drained["n"]]()
                    drained["n"] += 1
                    n += 1

            units = (
                [(0, 0, jc) for jc in range(8)]
                + [(1, 0, jc) for jc in range(8)]
                + [(0, 1, jc) for jc in range(16)]
                + [(1, 1, jc) for jc in range(16)]
            )
            # Forced drains serve two purposes: (a) PE program order must see
            # a filler's matmuls before any attention matmul that (via a
            # copy) depends on them, and (b) the fillers' PSUM->SBUF copies
            # must clear the VectorE FIFO well before a sweep boundary, or
            # the next sweep's q/k copies queue behind the norm chain and
            # stall the whole pipeline (13.9us gaps in the v2 trace).
            # Filler indices: Uv(2..7)=0..5, qk(p1,lb0)=6,7, qk(p0,lb1)=8,9,
            # Uv(8..15)=10..17, qk(p1,lb1)=18,19.
            barriers = {}
            for jc in range(1, 8):
                barriers[(0, 0, jc)] = jc  # Uv(jc-1) at filler jc-1
            barriers[(0, 0, 5)] = 10  # qk(p1,lb0) early, ahead of the boundary
            barriers[(1, 0, 2)] = 12  # qk(p0,lb1) early
            for jc in range(9, 16):
                barriers[(0, 1, jc)] = 12 + jc - 8  # Uv(jc-1) at 12+(jc-1-8)
            barriers[(0, 1, 8)] = 22  # all QKV in the stream by mid-sweep
            # The sweep-end flush emits the last AV (von chunk 7 or 15):
            flush_drain = {(0, 0): 8, (1, 0): 8, (0, 1): 20, (1, 1): 22}

            from collections import deque

            # AVs trail their unit by TWO units: the exp->mask->AV chain
            # (and any norm work inserted into the DVE/GpSimd FIFOs) gets
            # 2-4us of slack before the in-order PE reaches the AV, instead
            # of stalling the whole pipe behind it.
            pending_av = deque()
            po2_cur = [None]
            # Deferred actions: [countdown, fn] run at the top of a unit
            # once countdown units have passed. Norm phase 2 runs two units
            # after phase 1 so its PE matmul never stalls on the DVE chain,
            # yet still lands before the recycled-slot AVs (trail = 2 units).
            delayed = []

            for u in units:
                p, sweep, jc = u
                if u in barriers:
                    drain_until(barriers[u])
                if jc == 0:
                    # 4 x [66, 512] PSUM (one bank each): [hl][half]. With
                    # bufs=4 and a fixed allocation order, each tile recycles
                    # the same-(hl,half) tile of the previous sweep.
                    t = [
                        op_.tile([66, 512], f32, tag="po", name="po")
                        for _ in range(4)
                    ]
                    po2_cur[0] = [[t[0], t[1]], [t[2], t[3]]]
                # Deferred norm2 actions: their muls must be in the stream
                # before the AVs that overwrite the recycled PSUM halves.
                for d in delayed:
                    d[0] -= 1
                ready = [d for d in delayed if d[0] <= 0]
                delayed[:] = [d for d in delayed if d[0] > 0]
                for d in ready:
                    d[1]()
                win0 = 1024 * sweep
                j0 = jc * P
                a0 = max(j0, win0)
                w = win0 + 1024 - a0
                # 512-block-aligned chunks: per chunk, BOTH heads' scores
                # matmuls are emitted back-to-back so their K=64 row-group
                # halves run concurrently in the PE array; each chunk then
                # gets one exp + (after the trail) one AV into its po half.
                cs = a0
                while cs < win0 + 1024:
                    ce = min((cs // 512 + 1) * 512, win0 + 1024)
                    cw = ce - cs
                    qg = cs // 512
                    half = qg - 2 * sweep
                    pss = []
                    for hl in range(2):
                        hp = slice(hl * 64, (hl + 1) * 64)
                        ps = sp.tile([P, 512], f32, tag="ps")
                        nc.tensor.matmul(
                            ps[:, 0:cw],
                            kT[p][hp, j0 : j0 + P],
                            qT[p][hp, cs:ce],
                            start=True,
                            stop=True,
                        )
                        pss.append(ps)
                    for hl in range(2):
                        et = ep.tile([P, 512], bf16, tag="e")
                        nc.scalar.activation(
                            et[:, 0:cw], pss[hl][:, 0:cw], AF.Exp, scale=0.125
                        )
                        if cs == a0 and a0 == j0:
                            # diagonal block: zero where j > i
                            nc.gpsimd.tensor_mul(et[:, :P], et[:, :P], trimask[:])
                        if len(pending_av) >= 6:
                            emit_av(pending_av.popleft())
                        pending_av.append(
                            (
                                2 * p + hl,
                                po2_cur[0][hl][half],
                                et,
                                jc,
                                cs - win0 - half * 512,
                                cw,
                                qg,
                            )
                        )
                    cs = ce
                    if u == (0, 0, 0) and ce == 512:
                        # second q/k halves of pair 0: emitted between the
                        # first unit's chunks so the first exp fires earlier
                        emit_qk_half(0, "q", 0, 1, "s")
                        emit_qk_half(0, "k", 0, 1, "s")
                est["pe"] += 1.25 * w
                est["act"] = max(est["act"], est["pe"]) + 2 * (w + 344) / 1.2
                if u == (0, 1, 15):
                    reserve[0] = 0  # release the held-back fillers
                if jc == 8 * sweep + 6:
                    # the low i-half's last AV (from jc=3/11) has cleared the
                    # 6-chunk trail: normalize it mid-sweep so the boundary
                    # only handles the high half.
                    zfs = emit_norm1(po2_cur[0], 0)
                    delayed.append(
                        [2, lambda p=p, s=sweep, po2=po2_cur[0], z=zfs: (
                            emit_norm2(p, s, 0, po2, z))]
                    )
                if jc == (7 if sweep == 0 else 15):
                    # sweep end: flush the AV trail and start the high half's
                    # norm chain (apply deferred two units into next sweep).
                    drain_until(flush_drain[(p, sweep)])
                    while pending_av:
                        emit_av(pending_av.popleft())
                    zfs = emit_norm1(po2_cur[0], 1)
                    delayed.append(
                        [2, lambda p=p, s=sweep, po2=po2_cur[0], z=zfs: (
                            emit_norm2(p, s, 1, po2, z))]
                    )
                    if (p, sweep) == (1, 0):
                        # first-half output projection: emittable once
                        # norm2(1,0,hi) lands (delay 3 > the norm2's 2)
                        delayed.append(
                            [3, lambda: fillers.extend(
                                [lambda oc=oc: emit_oproj(oc, 0, "v")
                                 for oc in range(EC)])]
                        )
                    # bridge the PE over the norm latency with fillers
                    for _ in range(4):
                        if drained["n"] < len(fillers):
                            fillers[drained["n"]]()
                            drained["n"] += 1
                drain_budget()

            # tail: run remaining deferred actions, then the second l-half
            # output projection (copies split across ScalarE/VectorE).
            for d in sorted(delayed, key=lambda d: d[0]):
                d[1]()
            drain_until(len(fillers))
            for oc in range(EC):
                emit_oproj(oc, 1, "s" if oc % 2 == 0 else "v")
            if DEBUG_TAPS:
                for p in range(2):
                    nc.sync.dma_start(dbg["qTo"][p * P : (p + 1) * P, :], qT[p][:])
                    nc.sync.dma_start(dbg["kTo"][p * P : (p + 1) * P, :], kT[p][:])
                    nc.sync.dma_start(dbg["oTo"][p * P : (p + 1) * P, :], oT[p][:])
                nc.sync.dma_start(dbg["vono"][:], von[:])

    nc.compile()
    return nc


def make_in_maps(x, Wa, Wout_w, Wout_b):
    """Host-side sharding: per-core input dicts."""
    x = np.asarray(x, dtype=np.float32)
    Wa = np.asarray(Wa, dtype=np.float32)
    Wout_w = np.asarray(Wout_w, dtype=np.float32)
    b16 = ml_dtypes.bfloat16

    xTs = [np.ascontiguousarray(x[b].T).astype(b16) for b in range(B)]
    in_maps = []
    for c in range(N_CORES):
        b, hg = divmod(c, 4)
        heads = list(range(4 * hg, 4 * hg + 4))
        qrows = np.concatenate([Wa[192 * h : 192 * h + 64] for h in heads], 0)
        krows = np.concatenate([Wa[192 * h + 64 : 192 * h + 128] for h in heads], 0)
        vrows = np.concatenate([Wa[192 * h + 128 : 192 * h + 192] for h in heads], 0)
        waT = np.ascontiguousarray(
            np.concatenate([qrows, krows, vrows], 0).T
        ).astype(b16)
        woT = np.ascontiguousarray(
            np.concatenate([Wout_w[:, 64 * h : 64 * h + 64] for h in heads], 1).T
        ).astype(b16)
        in_maps.append({"xT": xTs[b], "waT": waT, "woT": woT})
    return in_maps


def combine_outputs(core_outs, Wout_b):
    """core_outs: list of 8 outT [E, L] partials -> full [B, L, E]."""
    Wout_b = np.asarray(Wout_b, dtype=np.float32)
    out = np.empty((B, L, E), np.float32)
    for b in range(B):
        acc = np.asarray(core_outs[4 * b], np.float32)
        for c in range(4 * b + 1, 4 * b + 4):
            acc = acc + np.asarray(core_outs[c], np.float32)
        out[b] = acc.T + Wout_b[None, :]
    return out


def kernel(x, Wa, Wout_w, Wout_b):
    nc = build_nc()
    in_maps = make_in_maps(x, Wa, Wout_w, Wout_b)
    res = run_bass_kernel_spmd(nc, in_maps, list(range(N_CORES)))
    return combine_outputs([r["outT"] for r in res.results], Wout_b)


if __name__ == "__main__":
    rng = np.random.default_rng(0)
    x = rng.standard_normal((B, L, E), dtype=np.float32)
    Wa = rng.standard_normal((3 * H * D, E), dtype=np.float32) * 0.02
    Ww = rng.standard_normal((E, H * D), dtype=np.float32) * 0.02
    Wb = rng.standard_normal((E,), dtype=np.float32) * 0.02
    out = kernel(x, Wa=Wa, Wout_w=Ww, Wout_b=Wb)
    print(out.shape, out.dtype)


# revision 51
# speedup vs baseline: 1.1560x; 1.1560x over previous
"""Multi-head causal attention (B=2, L=2048, E=1024, H=16, D=64) on 8 NeuronCores.

Sharding: data-parallel over batch x tensor-parallel over heads.
  core c: batch b = c // 4, head group hg = c % 4 -> heads [4*hg, 4*hg+4).
Each core computes QKV projection for its 4 heads, causal softmax attention,
and a *partial* output projection (its heads' slice of Wout). The host sums
the 4 partial outputs per batch and adds the bias.

Device design notes (v2):
  - Matmul operands are bf16 (fp32 PSUM accumulation); host pre-casts inputs.
  - Host pre-transposes everything so the device never transposes:
      xT   [E, L]   = x[b].T                      (bf16)
      waT  [E, 768] = Wa rows regrouped [q_h0..q_h3 | k_h0.. | v_h0..].T (bf16)
      woT  [256, E] = Wout_w columns for this core's heads, transposed   (bf16)
  - Attention runs in the S^T layout (scores[j, i]); softmax denominator Z
    comes from a ones-column appended to V (PSUM row 64; row 65 is pad).
  - No max-subtraction in softmax: scores are ~N(0, 0.41^2), exp can't overflow.
  - The attention middle is ScalarE(exp)-bound (~72us), so all other PE work
    (second-half QKV projection, output projection) is drip-fed into the
    attention stream via a cycle-budget greedy to keep the PE dense and the
    HAM clock-gate at 8/8. A warmup matmul burst during the input DMA trips
    the HAM to full clock before the QKV projection starts.
  - ScalarE runs ONLY exp (plus 4 early copies before the first exp); all
    other PSUM->SBUF moves go to VectorE, the causal-mask multiplies go to
    GpSimd, 1/Z is a direct-from-PSUM fast reciprocal, and the per-column
    1/Z broadcast is an SBUF->SBUF partition-broadcast DMA (no PE/DVE cost).
  - Output projection accumulates both head-pairs in one PSUM tile (no
    HBM read-modify-write accumulation, half the output DMA).
"""

import ml_dtypes
import numpy as np

import concourse.bass as bass
import concourse.mybir as mybir
import concourse.tile as tile
from concourse import bacc
from concourse.bass_utils import run_bass_kernel_spmd
from concourse.masks import make_upper_triangular


P = 128
B = 2
L = 2048
E = 1024
H = 16
D = 64
HC = 4            # heads per core
F = HC * D        # 256: this core's slice of the head dim
EC = E // P       # 8 chunks of the embed dim
NLC = L // P      # 16 l-chunks
VST = NLC * 66    # v stride per head: 16 chunks of [64 v | 1 ones | 1 pad]

f32 = mybir.dt.float32
f32r = mybir.dt.float32r
bf16 = mybir.dt.bfloat16
AF = mybir.ActivationFunctionType
N_CORES = 8


DEBUG_TAPS = False


def build_nc():
    nc = bacc.Bacc(None, target_bir_lowering=False, debug=False)

    xT = nc.dram_tensor("xT", [E, L], bf16, kind="ExternalInput")
    waT = nc.dram_tensor("waT", [E, 3 * F], bf16, kind="ExternalInput")
    woT = nc.dram_tensor("woT", [F, E], bf16, kind="ExternalInput")
    outT = nc.dram_tensor("outT", [E, L], bf16, kind="ExternalOutput")
    if DEBUG_TAPS:
        dbg = {
            "qTo": nc.dram_tensor("qTo", [2 * P, L], bf16, kind="ExternalOutput"),
            "kTo": nc.dram_tensor("kTo", [2 * P, L], bf16, kind="ExternalOutput"),
            "vono": nc.dram_tensor(
                "vono", [P, HC * VST], bf16, kind="ExternalOutput"
            ),
            "oTo": nc.dram_tensor("oTo", [2 * P, L], bf16, kind="ExternalOutput"),
        }

    with tile.TileContext(nc) as tc:
        with (
            tc.tile_pool(name="persist", bufs=1) as pp,
            tc.tile_pool(name="qkv", bufs=1) as qp,
            tc.tile_pool(name="xw", bufs=1) as xp,
            tc.tile_pool(name="sps", bufs=4, space="PSUM") as sp,
            tc.tile_pool(name="ops", bufs=4, space="PSUM") as op_,
            tc.tile_pool(name="epool", bufs=8) as ep,
            tc.tile_pool(name="npool", bufs=2) as npl,
            tc.tile_pool(name="ob", bufs=3) as ob,
        ):
            # Persistent SBUF tensors.
            qT = [qp.tile([P, L], bf16, tag=f"q{p}", name=f"qT{p}") for p in range(2)]
            kT = [qp.tile([P, L], bf16, tag=f"k{p}", name=f"kT{p}") for p in range(2)]
            von = qp.tile([P, HC * VST], bf16, tag="von", name="von")
            oT = [qp.tile([P, L], bf16, tag=f"o{p}", name=f"oT{p}") for p in range(2)]
            wo_sb = [
                pp.tile([P, E], bf16, tag=f"wo{fc}", name=f"wo{fc}") for fc in range(2)
            ]
            onesf = pp.tile([P, 64], f32, tag="onesf")
            ones1 = pp.tile([1, 64], f32r, tag="ones1")
            warm_w = pp.tile([P, 64], bf16, tag="warm")
            trimask = pp.tile([P, P], bf16, tag="trimask")
            trimaskf = pp.tile([P, P], f32, tag="trimaskf")

            x_sb = [
                xp.tile([P, L], bf16, tag=f"x{ec}", name=f"x{ec}") for ec in range(EC)
            ]
            wa_sb = [
                xp.tile([P, 3 * F], bf16, tag=f"wa{ec}", name=f"wa{ec}")
                for ec in range(EC)
            ]

            # Input DMAs in priority order: the l-half-0 x plus the q/k
            # sections of Wa unblock the first QKV tiles ~10us in; v/x-lb1/wo
            # stream behind.
            for ec in range(EC):
                nc.sync.dma_start(x_sb[ec][:, 0:1024], xT[ec * P : (ec + 1) * P, 0:1024])
                nc.sync.dma_start(
                    wa_sb[ec][:, 0 : 2 * F], waT[ec * P : (ec + 1) * P, 0 : 2 * F]
                )
            for ec in range(EC):
                nc.sync.dma_start(
                    wa_sb[ec][:, 2 * F : 3 * F],
                    waT[ec * P : (ec + 1) * P, 2 * F : 3 * F],
                )
            for ec in range(EC):
                nc.sync.dma_start(
                    x_sb[ec][:, 1024:2048], xT[ec * P : (ec + 1) * P, 1024:2048]
                )
            for fc in range(2):
                nc.sync.dma_start(wo_sb[fc][:], woT[fc * P : (fc + 1) * P, :])

            # Constants (memset/affine_select can't encode bf16: build f32, cast)
            nc.gpsimd.memset(onesf[:], 1.0)
            nc.vector.tensor_copy(warm_w[:], onesf[:])
            nc.vector.tensor_copy(ones1[:], onesf[0:1, :])
            make_upper_triangular(nc, trimaskf[:], val=1.0, diag=True)
            nc.vector.tensor_copy(trimask[:], trimaskf[:])
            # ones/pad columns of von (Z rows): cols [64:66] of each 66-chunk
            for h in range(HC):
                dst = von[:].rearrange("p (g n t) -> p g n t", g=HC, t=66)[
                    :, h, :, 64:66
                ]
                nc.vector.tensor_copy(
                    dst, onesf[:, 0:32].rearrange("p (n t) -> p n t", t=2)
                )

            # HAM warmup: ~7us of junk matmuls spanning the input-DMA wait, so
            # the PE clock-gate is at 8/8 when the real work starts and never
            # sees a >3.4us idle window at the kernel head.
            wps = sp.tile([P, 512], f32, tag="ps")
            for i in range(128):
                nc.tensor.matmul(
                    wps[0:64, 0:64], warm_w[:], warm_w[:], start=True, stop=True
                )

            # ---------------- emitters ----------------
            est = {"pe": 0.0, "act": 0.0}

            def emit_qk(p, which, lb, eng):
                off = 0 if which == "q" else F
                dst = (qT if which == "q" else kT)[p]
                cp = nc.scalar.copy if eng == "s" else nc.vector.tensor_copy
                for s in range(2):
                    ps = sp.tile([P, 512], f32, tag="ps")
                    for ec in range(EC):
                        nc.tensor.matmul(
                            ps[:],
                            wa_sb[ec][:, off + p * P : off + (p + 1) * P],
                            x_sb[ec][:, lb * 1024 + s * 512 : lb * 1024 + (s + 1) * 512],
                            start=(ec == 0),
                            stop=(ec == EC - 1),
                        )
                    cp(
                        dst[:, lb * 1024 + s * 512 : lb * 1024 + (s + 1) * 512],
                        ps[:],
                    )
                est["pe"] += 3413

            def emit_v(lc, eng):
                # v natural [l, d] for all 4 heads at once (free dim 256)
                ps = sp.tile([P, 512], f32, tag="ps")
                for ec in range(EC):
                    nc.tensor.matmul(
                        ps[:, 0:F],
                        x_sb[ec][:, lc * P : (lc + 1) * P],
                        wa_sb[ec][:, 2 * F : 3 * F],
                        start=(ec == 0),
                        stop=(ec == EC - 1),
                    )
                dst = von[:].rearrange("p (g c) -> p g c", g=HC)[
                    :, :, lc * 66 : lc * 66 + 64
                ]
                src = ps[:, 0:F].rearrange("p (g c) -> p g c", g=HC)
                cp = nc.scalar.copy if eng == "s" else nc.vector.tensor_copy
                cp(dst, src)
                est["pe"] += 853

            def emit_av(u):
                # One chunk = one 512-aligned block of i: a single matmul
                # into the [hl][half] po accumulator.
                h, po_half, et, jc, off, cw, qg = u
                nc.tensor.matmul(
                    po_half[:, off : off + cw],
                    von[:, h * VST + jc * 66 : h * VST + (jc + 1) * 66],
                    et[:, 0:cw],
                    start=(jc == 0),
                    stop=(jc == 4 * qg + 3),
                )

            def emit_norm1(po2, half):
                # phase 1 of normalization for one 512-wide i-half: 1/Z into
                # SBUF (VectorE). reciprocal_approx_fast reads garbage from
                # PSUM on HW: bounce the Z row through SBUF first.
                zfs = []
                for hl in range(2):
                    zc = npl.tile([1, 512], f32, tag="zc")
                    nc.vector.tensor_copy(zc[:], po2[hl][half][64:65, :])
                    zf = npl.tile([1, 512], f32, tag="zf")
                    nc.vector.reciprocal_approx_fast(zf[:], zc[:])
                    zfs.append(zf)
                return zfs

            def emit_norm2(p, sweep, half, po2, zfs):
                # phase 2: replicate 1/Z across 64 partitions with a K=1
                # ones-outer-product matmul (PE; partition_broadcast on
                # GpSimd forces a ~6us Q7 library swap around every call),
                # bounce to SBUF, and apply.
                win = slice(
                    sweep * 1024 + half * 512, sweep * 1024 + (half + 1) * 512
                )
                for hl in range(2):
                    zr = npl.tile([1, 512], f32r, tag="zr")
                    nc.vector.tensor_copy(zr[:], zfs[hl][:])
                    zp = sp.tile([P, 512], f32, tag="ps")
                    nc.tensor.matmul(
                        zp[0:64, 0:512], ones1[:], zr[:], start=True, stop=True
                    )
                    zs = npl.tile([64, 512], f32, tag="zs")
                    nc.vector.tensor_copy(zs[:], zp[0:64, 0:512])
                    nc.vector.tensor_mul(
                        oT[p][hl * 64 : (hl + 1) * 64, win],
                        po2[hl][half][0:64, :],
                        zs[:],
                    )

            def emit_oproj(oc, lb, eng):
                # output projection for out-chunk oc, l-half lb: both head
                # pairs accumulate in one PSUM tile (K=128 each).
                cp = nc.scalar.copy if eng == "s" else nc.vector.tensor_copy
                ot = ob.tile([P, 1024], bf16, tag="ot")
                for s in range(2):
                    ps = sp.tile([P, 512], f32, tag="ps")
                    for fc in range(2):
                        nc.tensor.matmul(
                            ps[:],
                            wo_sb[fc][:, oc * P : (oc + 1) * P],
                            oT[fc][:, lb * 1024 + s * 512 : lb * 1024 + (s + 1) * 512],
                            start=(fc == 0),
                            stop=(fc == 1),
                        )
                    cp(ot[:, s * 512 : (s + 1) * 512], ps[:])
                nc.sync.dma_start(
                    outT[oc * P : (oc + 1) * P, lb * 1024 : (lb + 1) * 1024], ot[:]
                )
                est["pe"] += 853

            # ---------------- schedule ----------------
            # Opening: just enough QKV for the first attention units, copies
            # on ScalarE (idle until the first exp).
            emit_qk(0, "q", 0, "s")
            emit_qk(0, "k", 0, "s")
            emit_v(0, "s")
            emit_v(1, "s")

            # Filler list (order matters: consumed FIFO; barriers below index
            # into it). All remaining QKV + (appended later) the first-half
            # output projection.
            fillers = []
            fillers += [lambda lc=lc: emit_v(lc, "v") for lc in range(2, 8)]  # 0..5
            fillers += [
                lambda: emit_qk(1, "q", 0, "v"),  # 6
                lambda: emit_qk(1, "k", 0, "v"),  # 7
                lambda: emit_qk(0, "q", 1, "v"),  # 8
                lambda: emit_qk(0, "k", 1, "v"),  # 9
            ]
            fillers += [lambda lc=lc: emit_v(lc, "v") for lc in range(8, 16)]  # 10..17
            fillers += [
                lambda: emit_qk(1, "q", 1, "v"),  # 18
                lambda: emit_qk(1, "k", 1, "v"),  # 19
            ]
            drained = {"n": 0}

            def drain_until(k):
                while drained["n"] < k:
                    fillers[drained["n"]]()
                    drained["n"] += 1

            reserve = [6]  # fillers held back for the final (filler-less) sweep

            def drain_budget():
                # Emit fillers while the PE would otherwise idle behind ACT,
                # but at most 2 per unit so the supply lasts to the final
                # (pure-attention) sweeps instead of front-loading.
                n = 0
                while (
                    drained["n"] < len(fillers) - reserve[0]
                    and est["act"] - est["pe"] > 400
                    and n < 2
                ):
                    fillers[drained["n"]]()
                    drained["n"] += 1
                    n += 1

            units = (
                [(0, 0, jc) for jc in range(8)]
                + [(1, 0, jc) for jc in range(8)]
                + [(0, 1, jc) for jc in range(16)]
                + [(1, 1, jc) for jc in range(16)]
            )
            # Forced drains serve two purposes: (a) PE program order must see
            # a filler's matmuls before any attention matmul that (via a
            # copy) depends on them, and (b) the fillers' PSUM->SBUF copies
            # must clear the VectorE FIFO well before a sweep boundary, or
            # the next sweep's q/k copies queue behind the norm chain and
            # stall the whole pipeline (13.9us gaps in the v2 trace).
            # Filler indices: Uv(2..7)=0..5, qk(p1,lb0)=6,7, qk(p0,lb1)=8,9,
            # Uv(8..15)=10..17, qk(p1,lb1)=18,19.
            barriers = {}
            for jc in range(3, 8):
                barriers[(0, 0, jc)] = jc - 2  # Uv(jc-1) at filler jc-3
            barriers[(0, 0, 5)] = 7  # qk(p1,lb0) early, ahead of the boundary
            barriers[(0, 0, 6)] = 8
            barriers[(1, 0, 2)] = 10  # qk(p0,lb1) early
            for jc in range(9, 16):
                barriers[(0, 1, jc)] = 10 + jc - 8  # Uv(jc-1) at 10+(jc-1-8)
            barriers[(0, 1, 8)] = 20  # all QKV in the stream by mid-sweep
            # The sweep-end flush emits the last AV (von chunk 7 or 15):
            flush_drain = {(0, 0): 6, (1, 0): 8, (0, 1): 18, (1, 1): 20}

            from collections import deque

            # AVs trail their unit by TWO units: the exp->mask->AV chain
            # (and any norm work inserted into the DVE/GpSimd FIFOs) gets
            # 2-4us of slack before the in-order PE reaches the AV, instead
            # of stalling the whole pipe behind it.
            pending_av = deque()
            po2_cur = [None]
            # Deferred actions: [countdown, fn] run at the top of a unit
            # once countdown units have passed. Norm phase 2 runs two units
            # after phase 1 so its PE matmul never stalls on the DVE chain,
            # yet still lands before the recycled-slot AVs (trail = 2 units).
            delayed = []

            for u in units:
                p, sweep, jc = u
                if u in barriers:
                    drain_until(barriers[u])
                if jc == 0:
                    # 4 x [66, 512] PSUM (one bank each): [hl][half]. With
                    # bufs=4 and a fixed allocation order, each tile recycles
                    # the same-(hl,half) tile of the previous sweep.
                    t = [
                        op_.tile([66, 512], f32, tag="po", name="po")
                        for _ in range(4)
                    ]
                    po2_cur[0] = [[t[0], t[1]], [t[2], t[3]]]
                # Deferred norm2 actions: their muls must be in the stream
                # before the AVs that overwrite the recycled PSUM halves.
                for d in delayed:
                    d[0] -= 1
                ready = [d for d in delayed if d[0] <= 0]
                delayed[:] = [d for d in delayed if d[0] > 0]
                for d in ready:
                    d[1]()
                win0 = 1024 * sweep
                j0 = jc * P
                a0 = max(j0, win0)
                w = win0 + 1024 - a0
                # 512-block-aligned chunks: per chunk, BOTH heads' scores
                # matmuls are emitted back-to-back so their K=64 row-group
                # halves run concurrently in the PE array; each chunk then
                # gets one exp + (after the trail) one AV into its po half.
                cs = a0
                while cs < win0 + 1024:
                    ce = min((cs // 512 + 1) * 512, win0 + 1024)
                    cw = ce - cs
                    qg = cs // 512
                    half = qg - 2 * sweep
                    pss = []
                    for hl in range(2):
                        hp = slice(hl * 64, (hl + 1) * 64)
                        ps = sp.tile([P, 512], f32, tag="ps")
                        nc.tensor.matmul(
                            ps[:, 0:cw],
                            kT[p][hp, j0 : j0 + P],
                            qT[p][hp, cs:ce],
                            start=True,
                            stop=True,
                        )
                        pss.append(ps)
                    for hl in range(2):
                        et = ep.tile([P, 512], bf16, tag="e")
                        nc.scalar.activation(
                            et[:, 0:cw], pss[hl][:, 0:cw], AF.Exp, scale=0.125
                        )
                        if cs == a0 and a0 == j0:
                            # diagonal block: zero where j > i
                            nc.gpsimd.tensor_mul(et[:, :P], et[:, :P], trimask[:])
                        if len(pending_av) >= 6:
                            emit_av(pending_av.popleft())
                        pending_av.append(
                            (
                                2 * p + hl,
                                po2_cur[0][hl][half],
                                et,
                                jc,
                                cs - win0 - half * 512,
                                cw,
                                qg,
                            )
                        )
                    cs = ce
                est["pe"] += 1.25 * w
                est["act"] = max(est["act"], est["pe"]) + 2 * (w + 344) / 1.2
                if u == (1, 1, 0):
                    reserve[0] = 0  # release the held-back fillers
                if jc == 8 * sweep + 6:
                    # the low i-half's last AV (from jc=3/11) has cleared the
                    # 6-chunk trail: normalize it mid-sweep so the boundary
                    # only handles the high half.
                    zfs = emit_norm1(po2_cur[0], 0)
                    delayed.append(
                        [2, lambda p=p, s=sweep, po2=po2_cur[0], z=zfs: (
                            emit_norm2(p, s, 0, po2, z))]
                    )
                if jc == (7 if sweep == 0 else 15):
                    # sweep end: flush the AV trail and start the high half's
                    # norm chain (apply deferred two units into next sweep).
                    drain_until(flush_drain[(p, sweep)])
                    while pending_av:
                        emit_av(pending_av.popleft())
                    zfs = emit_norm1(po2_cur[0], 1)
                    delayed.append(
                        [2, lambda p=p, s=sweep, po2=po2_cur[0], z=zfs: (
                            emit_norm2(p, s, 1, po2, z))]
                    )
                    if (p, sweep) == (1, 0):
                        # first-half output projection: emittable once
                        # norm2(1,0,hi) lands (delay 3 > the norm2's 2)
                        delayed.append(
                            [3, lambda: fillers.extend(
                                [lambda oc=oc: emit_oproj(oc, 0, "v")
                                 for oc in range(EC)])]
                        )
                    # bridge the PE over the norm latency with fillers
                    for _ in range(3):
                        if drained["n"] < len(fillers):
                            fillers[drained["n"]]()
                            drained["n"] += 1
                drain_budget()

            # tail: run remaining deferred actions, then the second l-half
            # output projection (copies split across ScalarE/VectorE).
            for d in sorted(delayed, key=lambda d: d[0]):
                d[1]()
            drain_until(len(fillers))
            for oc in range(EC):
                emit_oproj(oc, 1, "s" if oc % 2 == 0 else "v")
            if DEBUG_TAPS:
                for p in range(2):
                    nc.sync.dma_start(dbg["qTo"][p * P : (p + 1) * P, :], qT[p][:])
                    nc.sync.dma_start(dbg["kTo"][p * P : (p + 1) * P, :], kT[p][:])
                    nc.sync.dma_start(dbg["oTo"][p * P : (p + 1) * P, :], oT[p][:])
                nc.sync.dma_start(dbg["vono"][:], von[:])

    nc.compile()
    return nc


def make_in_maps(x, Wa, Wout_w, Wout_b):
    """Host-side sharding: per-core input dicts."""
    x = np.asarray(x, dtype=np.float32)
    Wa = np.asarray(Wa, dtype=np.float32)
    Wout_w = np.asarray(Wout_w, dtype=np.float32)
    b16 = ml_dtypes.bfloat16

    xTs = [np.ascontiguousarray(x[b].T).astype(b16) for b in range(B)]
    in_maps = []
    for c in range(N_CORES):
        b, hg = divmod(c, 4)
        heads = list(range(4 * hg, 4 * hg + 4))
        qrows = np.concatenate([Wa[192 * h : 192 * h + 64] for h in heads], 0)
        krows = np.concatenate([Wa[192 * h + 64 : 192 * h + 128] for h in heads], 0)
        vrows = np.concatenate([Wa[192 * h + 128 : 192 * h + 192] for h in heads], 0)
        waT = np.ascontiguousarray(
            np.concatenate([qrows, krows, vrows], 0).T
        ).astype(b16)
        woT = np.ascontiguousarray(
            np.concatenate([Wout_w[:, 64 * h : 64 * h + 64] for h in heads], 1).T
        ).astype(b16)
        in_maps.append({"xT": xTs[b], "waT": waT, "woT": woT})
    return in_maps


def combine_outputs(core_outs, Wout_b):
    """core_outs: list of 8 outT [E, L] partials -> full [B, L, E]."""
    Wout_b = np.asarray(Wout_b, dtype=np.float32)
    out = np.empty((B, L, E), np.float32)
    for b in range(B):
        acc = np.asarray(core_outs[4 * b], np.float32)
        for c in range(4 * b + 1, 4 * b + 4):
            acc = acc + np.asarray(core_outs[c], np.float32)
        out[b] = acc.T + Wout_b[None, :]
    return out


def kernel(x, Wa, Wout_w, Wout_b):
    nc = build_nc()
    in_maps = make_in_maps(x, Wa, Wout_w, Wout_b)
    res = run_bass_kernel_spmd(nc, in_maps, list(range(N_CORES)))
    return combine_outputs([r["outT"] for r in res.results], Wout_b)


if __name__ == "__main__":
    rng = np.random.default_rng(0)
    x = rng.standard_normal((B, L, E), dtype=np.float32)
    Wa = rng.standard_normal((3 * H * D, E), dtype=np.float32) * 0.02
    Ww = rng.standard_normal((E, H * D), dtype=np.float32) * 0.02
    Wb = rng.standard_normal((E,), dtype=np.float32) * 0.02
    out = kernel(x, Wa=Wa, Wout_w=Ww, Wout_b=Wb)
    print(out.shape, out.dtype)
